# revision 1
# baseline (speedup 1.0000x reference)
"""Bahdanau additive-attention kernel for Trainium2, data-parallel over
batch across 8 NeuronCores.

Per batch b:
    energy  = tanh(dec_proj[b] + enc[b] @ W_enc + b_score)   # (L, DEC)
    scores  = energy @ v                                     # (L,)
    alpha   = softmax(scores)
    att[b]  = alpha @ enc[b]                                 # (2E,)

On-device layout (per core, 8 batches):
  - enc is staged host-side as enc_t[b, e, l] so encT tiles [128e, 512l]
    feed the PE directly as the moving operand; W_enc (e, d) is the
    stationary operand in its natural layout. The matmul computes
    enc_projT (d on partitions), which makes dec_proj + b_score a
    per-partition bias fused into the ACT tanh.
  - scores = v . energyT via PE matvec accumulated over d-tiles.
  - softmax skips the max-subtraction: |scores| <= sum|v| = 32, safely
    inside the fp32 exp range, and softmax is shift-invariant. Raw
    scores are broadcast to 128 partitions with a K=1 ones-matmul, the
    Exp runs on the broadcast tile with accum_out giving the partial
    denominator replicated per partition, so the final 1/den needs no
    cross-partition traffic.
  - att^T accumulates per chunk with DVE multiply+reduce over the encT
    tiles still resident in SBUF (no second DMA pass); each chunk's
    accumulation overlaps the next chunk's matmuls.
  - startup is DMA-paced, so batch 0 chunk 0 runs k-major (4 psum
    groups per pass) with the dec_proj preamble matmuls interleaved in
    data-arrival order.
  - f32r (fp32 data, full-rate PE mode) is used for all N>=256 matmuls.
"""

import numpy as np
from contextlib import ExitStack

import concourse.bass as bass
import concourse.tile as tile
from concourse import mybir
from concourse.bass_utils import run_bass_kernel_spmd
from concourse.vector_clock import ScopedClock, VectorClock

N_CORES = 8
B, L, DEC, ENC2 = 64, 1024, 1024, 2048
BL = B // N_CORES  # batches per core
KT = ENC2 // 128   # contraction tiles over e
DT = DEC // 128    # d tiles
LC = 512           # l-chunk (one PSUM bank of f32)
NLC = L // LC

F32 = mybir.dt.float32
F32R = mybir.dt.float32r
AF = mybir.ActivationFunctionType
ALU = mybir.AluOpType
AX = mybir.AxisListType


def _patch_tile_drain():
    """Workarounds for this container's walrus build.

    1. The Tile tail drain carries one sem wait per touched proc; walrus
       rejects >2 on the CTRL encoding. Split the waits onto single-wait
       SP nops (SP executes in order, so the drain then needs none).
    2. Any instruction with 2+ sem waits can fail codegen (the matmul
       LW encoding holds a single wait). Split multi-wait instructions:
       excess waits move onto same-engine InstNoOp carriers inserted
       just before; engine program order makes this equivalent.
    """
    if getattr(tile.TileContext, "_drain_patched", False):
        return

    def _drain_and_barrier(self, tick_clock, wait_clock):
        vec = list(tick_clock.global_clock)
        n = len(vec)
        for i in range(n):
            if vec[i] <= 0:
                continue
            part = [0] * n
            part[i] = vec[i]
            nop_inst = self.nc.sync.nop(nofuse=True)
            wait_clock.add_sem_waits(
                nop_inst.ins, ScopedClock({None: VectorClock(part)})
            )
        self.nc.sync.drain()
        self.nc.all_engine_barrier()
        assert self.sems is not None
        popped = self.nc._tile_sem_poison_stack.pop()
        assert popped is self._sem_poison
        self.nc.clear_and_free_semaphores(list(self.sems.allocated().values()))
        self.nc.all_engine_barrier()

    tile.TileContext._drain_and_barrier = _drain_and_barrier

    import bass_rust

    orig_lower = tile.TileContext._lower_ordered_insts

    def _lower_with_wait_split(self, ordered):
        for insts in ordered.values():
            expanded = []
            for inst in insts:
                si = inst.sync_info
                waits = list(si.on_wait) if si and si.on_wait else []
                if len(waits) > 1:
                    for w in waits[:-1]:
                        nop = mybir.InstNoOp(
                            name=self.nc.get_next_instruction_name(),
                            engine=inst.engine,
                            bass_nofuse=True,
                            sync_info=bass_rust.SyncInfo(on_wait=[w], on_update=[]),
                        )
                        self.nc.register_instruction(nop)
                        expanded.append(nop)
                    inst.sync_info = bass_rust.SyncInfo(
                        on_wait=[waits[-1]],
                        on_update=list(si.on_update) if si.on_update else [],
                    )
                expanded.append(inst)
            insts[:] = expanded
        return orig_lower(self, ordered)

    tile.TileContext._lower_ordered_insts = _lower_with_wait_split
    tile.TileContext._drain_patched = True


def build_nc():
    _patch_tile_drain()
    nc = bass.Bass()
    enc_t = nc.declare_dram_parameter("enc_t", [BL, ENC2, L], F32R, isOutput=False)
    dec_kpb = nc.declare_dram_parameter("dec_kpb", [128, DT, BL], F32, isOutput=False)
    w_score = nc.declare_dram_parameter(
        "w_score", [DEC + ENC2, DEC], F32R, isOutput=False
    )
    b_mat = nc.declare_dram_parameter("b_mat", [128, DT], F32, isOutput=False)
    v_mat = nc.declare_dram_parameter("v_mat", [128, DT], F32R, isOutput=False)
    eye = nc.declare_dram_parameter("eye", [128, 128], F32, isOutput=False)
    ones = nc.declare_dram_parameter("ones", [1, 128], F32R, isOutput=False)
    att = nc.declare_dram_parameter("att", [BL, ENC2], F32, isOutput=True)

    with tile.TileContext(nc) as tc, ExitStack() as ctx:
        singles = ctx.enter_context(tc.tile_pool(name="singles", bufs=1))
        smalls = ctx.enter_context(tc.tile_pool(name="smalls", bufs=2))
        wdec_pool = ctx.enter_context(tc.tile_pool(name="wdec", bufs=32))
        enc_pool = ctx.enter_context(tc.tile_pool(name="enc", bufs=6))
        energy_pool = ctx.enter_context(tc.tile_pool(name="energy", bufs=3))
        wbc_pool = ctx.enter_context(tc.tile_pool(name="wbc", bufs=2))
        prod_pool = ctx.enter_context(tc.tile_pool(name="prod", bufs=2))
        ep_ps = ctx.enter_context(tc.tile_pool(name="ep_ps", bufs=4, space="PSUM"))
        sc_ps = ctx.enter_context(tc.tile_pool(name="sc_ps", bufs=1, space="PSUM"))
        wb_ps = ctx.enter_context(tc.tile_pool(name="wb_ps", bufs=1, space="PSUM"))
        dec_ps = ctx.enter_context(tc.tile_pool(name="dec_ps", bufs=1, space="PSUM"))
        att_ps_pool = ctx.enter_context(
            tc.tile_pool(name="att_ps", bufs=1, space="PSUM")
        )

        # ---- persistent tiles -------------------------------------------
        wenc = singles.tile([128, KT, DEC], F32R)  # W_enc, (e-tile, k) x d
        dec_sb = singles.tile([128, DT, BL], F32)
        b_sb = singles.tile([128, DT], F32)
        v_sb = singles.tile([128, DT], F32R)
        eye_sb = singles.tile([128, 128], F32)
        bias_sb = singles.tile([128, DT, BL], F32)  # dec_proj + b_score
        att_all = singles.tile([128, KT * BL], F32)  # att^T cols = b*KT+k
        ones_sb = singles.tile([1, 128], F32R)

        # ---- startup DMA, in data-arrival order -------------------------
        nc.sync.dma_start(out=dec_sb, in_=dec_kpb[:, :, :])
        nc.sync.dma_start(out=b_sb, in_=b_mat[:, :])
        nc.sync.dma_start(out=v_sb, in_=v_mat[:, :])
        nc.sync.dma_start(out=eye_sb, in_=eye[:, :])
        nc.sync.dma_start(out=ones_sb, in_=ones[:, :])

        # wd tiles dt-major: group k of the startup loop carries
        # (dt, kk) pairs [4k : 4k+4]
        wd_order = [(dt, kk) for dt in range(DT) for kk in range(DT)]
        wd_tiles = {}
        KH = KT // 2

        def alloc_chunk(nm):
            a = enc_pool.tile([128, KH, LC], F32R, tag="enc", name=f"{nm}a")
            bb = enc_pool.tile([128, KH, LC], F32R, tag="enc", name=f"{nm}b")
            return (a, bb)

        def enc_sl(ch, k, lo=0, width=LC, pair=False):
            t, kk = (ch[0], k) if k < KH else (ch[1], k - KH)
            if pair:
                return t[:, kk : kk + 2, lo : lo + width]
            return t[:, kk, lo : lo + width]

        enc00 = alloc_chunk("enc00")

        def emit_pre_mm(dt, kk, dpsum):
            nc.tensor.matmul(
                dpsum,
                lhsT=wd_tiles[(dt, kk)],
                rhs=dec_sb[:, kk, :],
                start=(kk == 0),
                stop=(kk == DT - 1),
            )

        # chunk (0,0) runs k-major over two 4-dt-group passes, with the
        # dec preamble matmuls interleaved in arrival order. Only the
        # dt<4 slice of W_dec loads inside the k-loop (keeps the per-k
        # DMA budget at the PE's consumption rate); dt>=4 loads after.
        def emit_wd_dma(dt, kk):
            wd = wdec_pool.tile([128, 128], F32, tag="wd", name=f"wd_{dt}_{kk}")
            nc.sync.dma_start(
                out=wd,
                in_=w_score[
                    kk * 128 : (kk + 1) * 128, dt * 128 : (dt + 1) * 128
                ].bitcast(F32),
            )
            wd_tiles[(dt, kk)] = wd

        def emit_pre_group(dt, dpsum):
            for kk in range(DT):
                emit_pre_mm(dt, kk, dpsum)
            nc.vector.tensor_scalar_add(
                out=bias_sb[:, dt, :], in0=dpsum, scalar1=b_sb[:, dt : dt + 1]
            )

        ps00 = {
            dt: ep_ps.tile([128, LC], F32, tag="ep", name=f"ps00_{dt}")
            for dt in range(4)
        }
        dpsum = dec_ps.tile([128, BL], F32)
        for k in range(KT):
            nc.sync.dma_start(
                out=enc_sl(enc00, k), in_=enc_t[0, k * 128 : (k + 1) * 128, 0:LC]
            )
            nc.sync.dma_start(
                out=wenc[:, k, :], in_=w_score[DEC + k * 128 : DEC + (k + 1) * 128, :]
            )
            for dt, kk in wd_order[k * 2 : (k + 1) * 2]:
                emit_wd_dma(dt, kk)
            for dt, kk in wd_order[k * 2 : (k + 1) * 2]:
                emit_pre_mm(dt, kk, dpsum)
                if kk == DT - 1:
                    nc.vector.tensor_scalar_add(
                        out=bias_sb[:, dt, :],
                        in0=dpsum,
                        scalar1=b_sb[:, dt : dt + 1],
                    )
            for dt in range(4):
                nc.tensor.matmul(
                    ps00[dt],
                    lhsT=wenc[:, k, dt * 128 : (dt + 1) * 128],
                    rhs=enc_sl(enc00, k),
                    start=(k == 0),
                    stop=(k == KT - 1),
                )
        for dt, kk in wd_order[KT * 2 :]:
            emit_wd_dma(dt, kk)

        def tanh_and_score(ps, sc, b, dt):
            energy = energy_pool.tile([128, LC], F32R, tag="energy")
            nc.scalar.activation(
                out=energy,
                in_=ps,
                func=AF.Tanh,
                bias=bias_sb[:, dt, b : b + 1],
                scale=1.0,
            )
            nc.tensor.matmul(
                sc,
                lhsT=v_sb[:, dt : dt + 1],
                rhs=energy,
                start=(dt == 0),
                stop=(dt == DT - 1),
            )

        sc00 = sc_ps.tile([1, LC], F32, tag="sc")
        for dt in range(4):
            tanh_and_score(ps00[dt], sc00, 0, dt)
        ps00b = {}
        for dt in range(4, DT):
            ps = ep_ps.tile([128, LC], F32, tag="ep", name=f"ps00b_{dt}")
            for k in range(KT):
                nc.tensor.matmul(
                    ps,
                    lhsT=wenc[:, k, dt * 128 : (dt + 1) * 128],
                    rhs=enc_sl(enc00, k),
                    start=(k == 0),
                    stop=(k == KT - 1),
                )
            ps00b[dt] = ps
        for dt in range(4, DT):
            emit_pre_group(dt, dpsum)
        for dt in range(4, DT):
            tanh_and_score(ps00b[dt], sc00, 0, dt)

        def load_chunk(b, c):
            ch = alloc_chunk(f"enc_{b}_{c}")
            for k in range(KT):
                nc.sync.dma_start(
                    out=enc_sl(ch, k),
                    in_=enc_t[b, k * 128 : (k + 1) * 128, c * LC : (c + 1) * LC],
                )
            return ch

        def kmajor_chunk(b, c, enc_tile):
            """Compute one chunk's scores consuming enc tiles in DMA
            arrival (k) order: dt 0-3 accumulate k-major across 4 psum
            groups, then dt 4-7 run dt-major at full speed."""
            ps = {
                dt: ep_ps.tile([128, LC], F32, tag="ep", name=f"km_{b}_{c}_{dt}")
                for dt in range(4)
            }
            for k in range(KT):
                for dt in range(4):
                    nc.tensor.matmul(
                        ps[dt],
                        lhsT=wenc[:, k, dt * 128 : (dt + 1) * 128],
                        rhs=enc_sl(enc_tile, k),
                        start=(k == 0),
                        stop=(k == KT - 1),
                    )
            sc = sc_ps.tile([1, LC], F32, tag="sc", name=f"km_sc_{b}_{c}")
            for dt in range(4):
                tanh_and_score(ps[dt], sc, b, dt)
            for dt in range(4, DT):
                p2 = ep_ps.tile([128, LC], F32, tag="ep", name=f"km2_{b}_{c}_{dt}")
                for k in range(KT):
                    nc.tensor.matmul(
                        p2,
                        lhsT=wenc[:, k, dt * 128 : (dt + 1) * 128],
                        rhs=enc_sl(enc_tile, k),
                        start=(k == 0),
                        stop=(k == KT - 1),
                    )
                tanh_and_score(p2, sc, b, dt)
            return sc

        def chunk_softmax_wacc(enc_tile, sc, b, first, tag, lo=0, width=LC):
            """Raw-score exp + weighted reduce for one finished (sub)chunk
            [lo, lo+width). Returns the per-partition denominator tile."""
            s_sb = smalls.tile([1, width], F32R, tag="ssb", name=f"ssb_{b}_{tag}")
            nc.scalar.copy(out=s_sb, in_=sc[:, lo : lo + width])
            wb = wb_ps.tile([128, width], F32, tag="wb", name=f"wb_{b}_{tag}")
            nc.tensor.matmul(wb, lhsT=ones_sb, rhs=s_sb, start=True, stop=True)
            w_bc = wbc_pool.tile([128, width], F32, tag="wbc", name=f"wbc_{b}_{tag}")
            den_c = smalls.tile([128, 1], F32, tag=f"den{tag}", name=f"den_{b}_{tag}")
            nc.scalar.activation(
                out=w_bc, in_=wb, func=AF.Exp, bias=0.0, scale=1.0, accum_out=den_c
            )
            atmp = None
            if not first:
                atmp = smalls.tile([128, KT], F32, tag="atmp", name=f"atmp_{b}_{tag}")
            # w_bc broadcast over a pair of k-tiles (0-stride middle dim)
            wb_pair = bass.AP(
                tensor=w_bc.tensor,
                offset=w_bc.offset,
                ap=[w_bc.ap[0], [0, 2], w_bc.ap[1]],
            )
            for k in range(0, KT, 2):
                col = b * KT + k
                prod = prod_pool.tile(
                    [128, 2, width], F32, tag="prod", name=f"prod_{b}_{tag}_{k}"
                )
                nc.vector.tensor_mul(
                    out=prod,
                    in0=enc_sl(enc_tile, k, lo, width, pair=True).bitcast(F32),
                    in1=wb_pair,
                )
                dst = att_all[:, col : col + 2] if first else atmp[:, k : k + 2]
                nc.vector.tensor_reduce(out=dst, in_=prod, axis=AX.X, op=ALU.add)
            if not first:
                cols = slice(b * KT, (b + 1) * KT)
                nc.vector.tensor_add(
                    out=att_all[:, cols], in0=att_all[:, cols], in1=atmp
                )
            return den_c

        def batch_epilogue(b, dens):
            """Normalize, transpose, and store one batch's attention row."""
            rden = smalls.tile([128, 1], F32, tag="rden")
            nc.vector.tensor_add(out=rden, in0=dens[0], in1=dens[1])
            for extra in dens[2:]:
                nc.vector.tensor_add(out=rden, in0=rden, in1=extra)
            nc.vector.reciprocal(out=rden, in_=rden)
            cols = slice(b * KT, (b + 1) * KT)
            nc.vector.tensor_scalar_mul(att_all[:, cols], att_all[:, cols], rden)
            att_bt = att_ps_pool.tile([KT, 128], F32, tag="abt")
            nc.tensor.transpose(att_bt, att_all[:, cols], eye_sb)
            att_sb = smalls.tile([KT, 128], F32, tag="asb")
            nc.vector.tensor_copy(out=att_sb, in_=att_bt)
            nc.sync.dma_start(
                out=att[b].rearrange("(k p) -> k p", p=128), in_=att_sb
            )

        # ---- main loop (chunk (0,0) already computed above) -------------
        dens = [chunk_softmax_wacc(enc00, sc00, 0, True, "0")]
        for b in range(BL):
            for c in range(NLC):
                if (b, c) == (0, 0):
                    continue
                enc_tile = load_chunk(b, c)
                if (b, c) == (BL - 1, NLC - 1):
                    # final chunk: two 256-wide halves, so most of the
                    # softmax+reduce tail overlaps the remaining matmuls
                    sc = sc_ps.tile([1, LC], F32, tag="sc")
                    for h in range(2):
                        lo, w = h * (LC // 2), LC // 2
                        for dt in range(DT):
                            ps = ep_ps.tile(
                                [128, w], F32, tag="ep", name=f"ps_f{h}_{dt}"
                            )
                            for k in range(KT):
                                nc.tensor.matmul(
                                    ps,
                                    lhsT=wenc[:, k, dt * 128 : (dt + 1) * 128],
                                    rhs=enc_sl(enc_tile, k, lo, w),
                                    start=(k == 0),
                                    stop=(k == KT - 1),
                                )
                            energy = energy_pool.tile(
                                [128, w], F32R, tag="energy", name=f"en_f{h}_{dt}"
                            )
                            nc.scalar.activation(
                                out=energy,
                                in_=ps,
                                func=AF.Tanh,
                                bias=bias_sb[:, dt, b : b + 1],
                                scale=1.0,
                            )
                            nc.tensor.matmul(
                                sc[:, lo : lo + w],
                                lhsT=v_sb[:, dt : dt + 1],
                                rhs=energy,
                                start=(dt == 0),
                                stop=(dt == DT - 1),
                            )
                        if h == 0:
                            dens.append(
                                chunk_softmax_wacc(
                                    enc_tile, sc, b, False, f"1h{h}", lo=lo, width=w
                                )
                            )
                        else:
                            # last sub-chunk: two 128-wide wacc pieces so
                            # only the final quarter's reduce is exposed
                            for q in range(2):
                                dens.append(
                                    chunk_softmax_wacc(
                                        enc_tile,
                                        sc,
                                        b,
                                        False,
                                        f"1h{h}q{q}",
                                        lo=lo + q * (w // 2),
                                        width=w // 2,
                                    )
                                )
                elif (b, c) in ((0, 1), (1, 0), (1, 1)):
                    # startup transient: consume tiles in arrival order
                    sc = kmajor_chunk(b, c, enc_tile)
                    dens.append(
                        chunk_softmax_wacc(enc_tile, sc, b, c == 0, str(c))
                    )
                else:
                    sc = sc_ps.tile([1, LC], F32, tag="sc")
                    for dt in range(DT):
                        ps = ep_ps.tile([128, LC], F32, tag="ep")
                        for k in range(KT):
                            nc.tensor.matmul(
                                ps,
                                lhsT=wenc[:, k, dt * 128 : (dt + 1) * 128],
                                rhs=enc_sl(enc_tile, k),
                                start=(k == 0),
                                stop=(k == KT - 1),
                            )
                        tanh_and_score(ps, sc, b, dt)
                    dens.append(
                        chunk_softmax_wacc(enc_tile, sc, b, c == 0, str(c))
                    )
            batch_epilogue(b, dens)
            dens = []

    return nc


def shard_inputs(dec_hidden, enc_output, W_score, b_score, v):
    """Full inputs -> per-core input maps (host-side layout staging)."""
    dec_hidden = np.ascontiguousarray(dec_hidden, dtype=np.float32)
    W_score = np.ascontiguousarray(W_score, dtype=np.float32)
    b_mat = np.ascontiguousarray(
        np.asarray(b_score, dtype=np.float32).reshape(DT, 128).T
    )
    v_mat = np.ascontiguousarray(np.asarray(v, dtype=np.float32).reshape(DT, 128).T)
    eye = np.eye(128, dtype=np.float32)

    in_maps = []
    for core in range(N_CORES):
        sl = slice(core * BL, (core + 1) * BL)
        # (L, BL, 2E) -> (BL, 2E, L)
        enc_t = np.ascontiguousarray(
            np.asarray(enc_output[:, sl, :], dtype=np.float32).transpose(1, 2, 0)
        )
        # (BL, DEC) -> [p, kt, b]
        dec_kpb = np.ascontiguousarray(
            dec_hidden[sl].T.reshape(DT, 128, BL).transpose(1, 0, 2)
        )
        in_maps.append(
            {
                "enc_t": enc_t,
                "ones": np.ones((1, 128), dtype=np.float32),
                "dec_kpb": dec_kpb,
                "w_score": W_score,
                "b_mat": b_mat,
                "v_mat": v_mat,
                "eye": eye,
            }
        )
    return in_maps


_NC_CACHE = None


def kernel(dec_hidden, enc_output, W_score, b_score, v):
    global _NC_CACHE
    if _NC_CACHE is None:
        _NC_CACHE = build_nc()
    nc = _NC_CACHE
    in_maps = shard_inputs(dec_hidden, enc_output, W_score, b_score, v)
    res = run_bass_kernel_spmd(nc, in_maps, list(range(N_CORES)))
    return np.concatenate([res.results[i]["att"] for i in range(N_CORES)], axis=0)



# revision 2
# speedup vs baseline: 1.4182x; 1.4182x over previous
"""Bahdanau additive-attention kernel for Trainium2, data-parallel over
batch across 8 NeuronCores.

Per batch b:
    energy  = tanh(dec_proj[b] + enc[b] @ W_enc + b_score)   # (L, DEC)
    scores  = energy @ v                                     # (L,)
    alpha   = softmax(scores)
    att[b]  = alpha @ enc[b]                                 # (2E,)

On-device layout (per core, 8 batches):
  - enc is staged host-side TWICE: enc8_t[b, e, l] in fp8e4m3 feeds the
    PE energy matmul in perf_mode=DoubleRow (two k-subtiles per
    instruction, measured ~2x over bf16/f32r at N=512), and encb_t in
    bf16 feeds the DVE attention reduce. W_enc is pre-scaled by 128 on
    the host so its values sit mid-range in e4m3; the tanh activation
    rescales the psum by 1/128.
  - dec_proj preamble runs in bf16 (error negligible vs fp8 energy).
  - scores = v . energyT via PE matvec in bf16 over d-tiles.
  - softmax skips the max-subtraction: |scores| <= sum|v| = 32, safely
    inside the fp32 exp range. Raw scores broadcast to 128 partitions
    with a K=1 ones-matmul; Exp runs on the broadcast tile (bf16 out)
    with accum_out giving the replicated denominator per partition.
  - att^T accumulates via fused scalar_tensor_tensor on DVE:
    accum_out[e,1] = sum_l enc_bf16[e,l] * exp_scores[l] in a single
    instruction per (k-tile, chunk); no separate mul+reduce pass.
  - startup is DMA-paced, so batch 0 chunk 0 runs k-major with the
    dec_proj preamble matmuls interleaved in data-arrival order.
"""

import numpy as np
import ml_dtypes
from contextlib import ExitStack

import concourse.bass as bass
import concourse.tile as tile
from concourse import mybir
from concourse.bass_utils import run_bass_kernel_spmd
from concourse.vector_clock import ScopedClock, VectorClock

N_CORES = 8
B, L, DEC, ENC2 = 64, 1024, 1024, 2048
BL = B // N_CORES  # batches per core
KT = ENC2 // 128   # contraction tiles over e
KP = KT // 2       # DoubleRow pairs over e
DT = DEC // 128    # d tiles
LC = 512           # l-chunk (one PSUM bank of f32)
NLC = L // LC
WSCALE = 128.0     # host-side W_enc scaling for fp8 range

F32 = mybir.dt.float32
F32R = mybir.dt.float32r
F8 = mybir.dt.float8e4
BF16 = mybir.dt.bfloat16
AF = mybir.ActivationFunctionType
ALU = mybir.AluOpType
PM = mybir.MatmulPerfMode


def _patch_tile_drain():
    """Workarounds for this container's walrus build.

    1. The Tile tail drain carries one sem wait per touched proc; walrus
       rejects >2 on the CTRL encoding. Split the waits onto single-wait
       SP nops (SP executes in order, so the drain then needs none).
    2. Any instruction with 2+ sem waits can fail codegen (the matmul
       LW encoding holds a single wait). Split multi-wait instructions:
       excess waits move onto same-engine InstNoOp carriers inserted
       just before; engine program order makes this equivalent.
    """
    if getattr(tile.TileContext, "_drain_patched", False):
        return

    def _drain_and_barrier(self, tick_clock, wait_clock):
        vec = list(tick_clock.global_clock)
        n = len(vec)
        for i in range(n):
            if vec[i] <= 0:
                continue
            part = [0] * n
            part[i] = vec[i]
            nop_inst = self.nc.sync.nop(nofuse=True)
            wait_clock.add_sem_waits(
                nop_inst.ins, ScopedClock({None: VectorClock(part)})
            )
        self.nc.sync.drain()
        self.nc.all_engine_barrier()
        assert self.sems is not None
        popped = self.nc._tile_sem_poison_stack.pop()
        assert popped is self._sem_poison
        self.nc.clear_and_free_semaphores(list(self.sems.allocated().values()))
        self.nc.all_engine_barrier()

    tile.TileContext._drain_and_barrier = _drain_and_barrier

    import bass_rust

    orig_lower = tile.TileContext._lower_ordered_insts

    def _lower_with_wait_split(self, ordered):
        for insts in ordered.values():
            expanded = []
            for inst in insts:
                si = inst.sync_info
                waits = list(si.on_wait) if si and si.on_wait else []
                if len(waits) > 1:
                    for w in waits[:-1]:
                        nop = mybir.InstNoOp(
                            name=self.nc.get_next_instruction_name(),
                            engine=inst.engine,
                            bass_nofuse=True,
                            sync_info=bass_rust.SyncInfo(on_wait=[w], on_update=[]),
                        )
                        self.nc.register_instruction(nop)
                        expanded.append(nop)
                    inst.sync_info = bass_rust.SyncInfo(
                        on_wait=[waits[-1]],
                        on_update=list(si.on_update) if si.on_update else [],
                    )
                expanded.append(inst)
            insts[:] = expanded
        return orig_lower(self, ordered)

    tile.TileContext._lower_ordered_insts = _lower_with_wait_split
    tile.TileContext._drain_patched = True


def build_nc():
    _patch_tile_drain()
    nc = bass.Bass()
    enc8_t = nc.declare_dram_parameter("enc8_t", [BL, ENC2, L], F8, isOutput=False)
    encb_t = nc.declare_dram_parameter("encb_t", [BL, ENC2, L], BF16, isOutput=False)
    wenc8 = nc.declare_dram_parameter("wenc8", [ENC2, DEC], F8, isOutput=False)
    w_dec = nc.declare_dram_parameter("w_dec", [DEC, DEC], BF16, isOutput=False)
    dec_kpb = nc.declare_dram_parameter("dec_kpb", [128, DT, BL], BF16, isOutput=False)
    b_mat = nc.declare_dram_parameter("b_mat", [128, DT], F32, isOutput=False)
    v_mat = nc.declare_dram_parameter("v_mat", [128, DT], BF16, isOutput=False)
    eye = nc.declare_dram_parameter("eye", [128, 128], F32, isOutput=False)
    ones = nc.declare_dram_parameter("ones", [1, 128], F32R, isOutput=False)
    att = nc.declare_dram_parameter("att", [BL, ENC2], F32, isOutput=True)

    with tile.TileContext(nc) as tc, ExitStack() as ctx:
        singles = ctx.enter_context(tc.tile_pool(name="singles", bufs=1))
        smalls = ctx.enter_context(tc.tile_pool(name="smalls", bufs=2))
        wdec_pool = ctx.enter_context(tc.tile_pool(name="wdec", bufs=32))
        enc8_pool = ctx.enter_context(tc.tile_pool(name="enc8", bufs=8))
        encb_pool = ctx.enter_context(tc.tile_pool(name="encb", bufs=8))
        energy_pool = ctx.enter_context(tc.tile_pool(name="energy", bufs=3))
        wbc_pool = ctx.enter_context(tc.tile_pool(name="wbc", bufs=2))
        prod_pool = ctx.enter_context(tc.tile_pool(name="prod", bufs=2))
        ep_ps = ctx.enter_context(tc.tile_pool(name="ep_ps", bufs=4, space="PSUM"))
        sc_ps = ctx.enter_context(tc.tile_pool(name="sc_ps", bufs=1, space="PSUM"))
        wb_ps = ctx.enter_context(tc.tile_pool(name="wb_ps", bufs=1, space="PSUM"))
        dec_ps = ctx.enter_context(tc.tile_pool(name="dec_ps", bufs=1, space="PSUM"))
        att_ps_pool = ctx.enter_context(
            tc.tile_pool(name="att_ps", bufs=1, space="PSUM")
        )

        # ---- persistent tiles -------------------------------------------
        wenc = singles.tile([128, KT, DEC], F8)  # W_enc*128, (e-tile, k) x d
        dec_sb = singles.tile([128, DT, BL], BF16)
        b_sb = singles.tile([128, DT], F32)
        v_sb = singles.tile([128, DT], BF16)
        eye_sb = singles.tile([128, 128], F32)
        bias_sb = singles.tile([128, DT, BL], F32)  # dec_proj + b_score
        att_all = singles.tile([128, KT * BL], F32)  # att^T cols = b*KT+k
        ones_sb = singles.tile([1, 128], F32R)

        # ---- startup DMA, in data-arrival order -------------------------
        nc.sync.dma_start(out=dec_sb, in_=dec_kpb[:, :, :])
        nc.sync.dma_start(out=b_sb, in_=b_mat[:, :])
        nc.sync.dma_start(out=v_sb, in_=v_mat[:, :])
        nc.sync.dma_start(out=eye_sb, in_=eye[:, :])
        nc.sync.dma_start(out=ones_sb, in_=ones[:, :])

        # wd tiles dt-major: startup step k carries (dt, kk) pairs
        # [2k : 2k+2]
        wd_order = [(dt, kk) for dt in range(DT) for kk in range(DT)]
        wd_tiles = {}
        KH = KT // 2

        def alloc_chunk(pool, dtype, nm):
            a = pool.tile([128, KH, LC], dtype, tag="enc", name=f"{nm}a")
            bb = pool.tile([128, KH, LC], dtype, tag="enc", name=f"{nm}b")
            return (a, bb)

        def enc_sl(ch, k, lo=0, width=LC):
            t, kk = (ch[0], k) if k < KH else (ch[1], k - KH)
            return t[:, kk, lo : lo + width]

        def enc_pair(ch, kp):
            """[128, 2, LC] DoubleRow rhs slice for pair (2kp, 2kp+1)."""
            k = 2 * kp
            t, kk = (ch[0], k) if k < KH else (ch[1], k - KH)
            return t[:, kk : kk + 2, :]

        def w_pair(kp, dt):
            """[128, 2, 128] DoubleRow lhsT slice."""
            return wenc[:, 2 * kp : 2 * kp + 2, dt * 128 : (dt + 1) * 128]

        enc00 = alloc_chunk(enc8_pool, F8, "enc00")

        def emit_pre_mm(dt, kk, dpsum):
            nc.tensor.matmul(
                dpsum,
                lhsT=wd_tiles[(dt, kk)],
                rhs=dec_sb[:, kk, :],
                start=(kk == 0),
                stop=(kk == DT - 1),
            )

        def emit_wd_dma(dt, kk):
            wd = wdec_pool.tile([128, 128], BF16, tag="wd", name=f"wd_{dt}_{kk}")
            nc.sync.dma_start(
                out=wd,
                in_=w_dec[kk * 128 : (kk + 1) * 128, dt * 128 : (dt + 1) * 128],
            )
            wd_tiles[(dt, kk)] = wd

        def emit_pre_group(dt, dpsum):
            for kk in range(DT):
                emit_pre_mm(dt, kk, dpsum)
            nc.vector.tensor_scalar_add(
                out=bias_sb[:, dt, :], in0=dpsum, scalar1=b_sb[:, dt : dt + 1]
            )

        # chunk (0,0) runs k-major (pair-major) over a 4-dt-group pass,
        # with the dec preamble matmuls interleaved in arrival order.
        ps00 = {
            dt: ep_ps.tile([128, LC], F32, tag="ep", name=f"ps00_{dt}")
            for dt in range(4)
        }
        dpsum = dec_ps.tile([128, BL], F32)
        for k in range(KT):
            nc.sync.dma_start(
                out=enc_sl(enc00, k), in_=enc8_t[0, k * 128 : (k + 1) * 128, 0:LC]
            )
            nc.sync.dma_start(
                out=wenc[:, k, :], in_=wenc8[k * 128 : (k + 1) * 128, :]
            )
            for dt, kk in wd_order[k * 2 : (k + 1) * 2]:
                emit_wd_dma(dt, kk)
            for dt, kk in wd_order[k * 2 : (k + 1) * 2]:
                emit_pre_mm(dt, kk, dpsum)
                if kk == DT - 1:
                    nc.vector.tensor_scalar_add(
                        out=bias_sb[:, dt, :],
                        in0=dpsum,
                        scalar1=b_sb[:, dt : dt + 1],
                    )
            if k % 2 == 1:
                kp = k // 2
                for dt in range(4):
                    nc.tensor.matmul(
                        ps00[dt],
                        lhsT=w_pair(kp, dt),
                        rhs=enc_pair(enc00, kp),
                        start=(kp == 0),
                        stop=(kp == KP - 1),
                        perf_mode=PM.DoubleRow,
                    )
        for dt, kk in wd_order[KT * 2 :]:
            emit_wd_dma(dt, kk)
        encb00 = alloc_chunk(encb_pool, BF16, "encb00")
        for k in range(KT):
            nc.sync.dma_start(
                out=enc_sl(encb00, k), in_=encb_t[0, k * 128 : (k + 1) * 128, 0:LC]
            )

        def tanh_and_score(ps, sc, b, dt):
            energy = energy_pool.tile([128, LC], BF16, tag="energy")
            nc.scalar.activation(
                out=energy,
                in_=ps,
                func=AF.Tanh,
                bias=bias_sb[:, dt, b : b + 1],
                scale=1.0 / WSCALE,
            )
            nc.tensor.matmul(
                sc,
                lhsT=v_sb[:, dt : dt + 1],
                rhs=energy,
                start=(dt == 0),
                stop=(dt == DT - 1),
            )

        sc00 = sc_ps.tile([1, LC], F32, tag="sc")
        for dt in range(4):
            tanh_and_score(ps00[dt], sc00, 0, dt)
        ps00b = {}
        for dt in range(4, DT):
            ps = ep_ps.tile([128, LC], F32, tag="ep", name=f"ps00b_{dt}")
            for kp in range(KP):
                nc.tensor.matmul(
                    ps,
                    lhsT=w_pair(kp, dt),
                    rhs=enc_pair(enc00, kp),
                    start=(kp == 0),
                    stop=(kp == KP - 1),
                    perf_mode=PM.DoubleRow,
                )
            ps00b[dt] = ps
        for dt in range(4, DT):
            emit_pre_group(dt, dpsum)
        for dt in range(4, DT):
            tanh_and_score(ps00b[dt], sc00, 0, dt)

        def load_chunk(b, c):
            ch8 = alloc_chunk(enc8_pool, F8, f"enc_{b}_{c}")
            for k in range(KT):
                nc.sync.dma_start(
                    out=enc_sl(ch8, k),
                    in_=enc8_t[b, k * 128 : (k + 1) * 128, c * LC : (c + 1) * LC],
                )
            chb = alloc_chunk(encb_pool, BF16, f"encb_{b}_{c}")
            for k in range(KT):
                nc.sync.dma_start(
                    out=enc_sl(chb, k),
                    in_=encb_t[b, k * 128 : (k + 1) * 128, c * LC : (c + 1) * LC],
                )
            return ch8, chb

        def kmajor_chunk(b, c, enc_tile):
            """Compute one chunk's scores consuming enc tiles in DMA
            arrival (pair) order: dt 0-3 accumulate pair-major across 4
            psum groups, then dt 4-7 run dt-major at full speed."""
            ps = {
                dt: ep_ps.tile([128, LC], F32, tag="ep", name=f"km_{b}_{c}_{dt}")
                for dt in range(4)
            }
            for kp in range(KP):
                for dt in range(4):
                    nc.tensor.matmul(
                        ps[dt],
                        lhsT=w_pair(kp, dt),
                        rhs=enc_pair(enc_tile, kp),
                        start=(kp == 0),
                        stop=(kp == KP - 1),
                        perf_mode=PM.DoubleRow,
                    )
            sc = sc_ps.tile([1, LC], F32, tag="sc", name=f"km_sc_{b}_{c}")
            for dt in range(4):
                tanh_and_score(ps[dt], sc, b, dt)
            for dt in range(4, DT):
                p2 = ep_ps.tile([128, LC], F32, tag="ep", name=f"km2_{b}_{c}_{dt}")
                for kp in range(KP):
                    nc.tensor.matmul(
                        p2,
                        lhsT=w_pair(kp, dt),
                        rhs=enc_pair(enc_tile, kp),
                        start=(kp == 0),
                        stop=(kp == KP - 1),
                        perf_mode=PM.DoubleRow,
                    )
                tanh_and_score(p2, sc, b, dt)
            return sc

        def dtmajor_chunk(b, c, enc_tile):
            sc = sc_ps.tile([1, LC], F32, tag="sc", name=f"dm_sc_{b}_{c}")
            for dt in range(DT):
                ps = ep_ps.tile([128, LC], F32, tag="ep", name=f"dm_{b}_{c}_{dt}")
                for kp in range(KP):
                    nc.tensor.matmul(
                        ps,
                        lhsT=w_pair(kp, dt),
                        rhs=enc_pair(enc_tile, kp),
                        start=(kp == 0),
                        stop=(kp == KP - 1),
                        perf_mode=PM.DoubleRow,
                    )
                tanh_and_score(ps, sc, b, dt)
            return sc

        def chunk_softmax_wacc(encb_tile, sc, b, first, tag):
            """Raw-score exp + fused weighted reduce for one finished
            chunk. Returns the per-partition denominator tile."""
            s_sb = smalls.tile([1, LC], F32R, tag="ssb", name=f"ssb_{b}_{tag}")
            nc.scalar.copy(out=s_sb, in_=sc)
            wb = wb_ps.tile([128, LC], F32, tag="wb", name=f"wb_{b}_{tag}")
            nc.tensor.matmul(wb, lhsT=ones_sb, rhs=s_sb, start=True, stop=True)
            w_bc = wbc_pool.tile([128, LC], BF16, tag="wbc", name=f"wbc_{b}_{tag}")
            den_c = smalls.tile([128, 1], F32, tag=f"den{tag}", name=f"den_{b}_{tag}")
            nc.scalar.activation(
                out=w_bc, in_=wb, func=AF.Exp, bias=0.0, scale=1.0, accum_out=den_c
            )
            atmp = None
            if not first:
                atmp = smalls.tile([128, KT], F32, tag="atmp", name=f"atmp_{b}_{tag}")
            for k in range(KT):
                col = b * KT + k
                prod = prod_pool.tile(
                    [128, LC], BF16, tag="prod", name=f"prod_{b}_{tag}_{k}"
                )
                dst = att_all[:, col : col + 1] if first else atmp[:, k : k + 1]
                nc.vector.scalar_tensor_tensor(
                    out=prod,
                    in0=enc_sl(encb_tile, k),
                    scalar=1.0,
                    in1=w_bc,
                    op0=ALU.mult,
                    op1=ALU.mult,
                    accum_out=dst,
                )
            if not first:
                cols = slice(b * KT, (b + 1) * KT)
                nc.vector.tensor_add(
                    out=att_all[:, cols], in0=att_all[:, cols], in1=atmp
                )
            return den_c

        def batch_epilogue(b, dens):
            """Normalize, transpose, and store one batch's attention row."""
            rden = smalls.tile([128, 1], F32, tag="rden")
            nc.vector.tensor_add(out=rden, in0=dens[0], in1=dens[1])
            for extra in dens[2:]:
                nc.vector.tensor_add(out=rden, in0=rden, in1=extra)
            nc.vector.reciprocal(out=rden, in_=rden)
            cols = slice(b * KT, (b + 1) * KT)
            nc.vector.tensor_scalar_mul(att_all[:, cols], att_all[:, cols], rden)
            att_bt = att_ps_pool.tile([KT, 128], F32, tag="abt")
            nc.tensor.transpose(att_bt, att_all[:, cols], eye_sb)
            att_sb = smalls.tile([KT, 128], F32, tag="asb")
            nc.vector.tensor_copy(out=att_sb, in_=att_bt)
            nc.sync.dma_start(
                out=att[b].rearrange("(k p) -> k p", p=128), in_=att_sb
            )

        # ---- main loop (chunk (0,0) already computed above) -------------
        dens = [chunk_softmax_wacc(encb00, sc00, 0, True, "0")]
        for b in range(BL):
            for c in range(NLC):
                if (b, c) == (0, 0):
                    continue
                enc_tile, encb_tile = load_chunk(b, c)
                if (b, c) in ((0, 1), (1, 0), (1, 1)):
                    # startup transient: consume tiles in arrival order
                    sc = kmajor_chunk(b, c, enc_tile)
                else:
                    sc = dtmajor_chunk(b, c, enc_tile)
                dens.append(
                    chunk_softmax_wacc(encb_tile, sc, b, c == 0, str(c))
                )
            batch_epilogue(b, dens)
            dens = []

    return nc


def shard_inputs(dec_hidden, enc_output, W_score, b_score, v):
    """Full inputs -> per-core input maps (host-side layout staging)."""
    dec_hidden = np.ascontiguousarray(dec_hidden, dtype=np.float32)
    W_score = np.asarray(W_score, dtype=np.float32)
    w_dec = np.ascontiguousarray(W_score[:DEC]).astype(ml_dtypes.bfloat16)
    wenc8 = np.ascontiguousarray(W_score[DEC:] * WSCALE).astype(
        ml_dtypes.float8_e4m3
    )
    b_mat = np.ascontiguousarray(
        np.asarray(b_score, dtype=np.float32).reshape(DT, 128).T
    )
    v_mat = (
        np.ascontiguousarray(np.asarray(v, dtype=np.float32).reshape(DT, 128).T)
        .astype(ml_dtypes.bfloat16)
    )
    eye = np.eye(128, dtype=np.float32)

    in_maps = []
    for core in range(N_CORES):
        sl = slice(core * BL, (core + 1) * BL)
        # (L, BL, 2E) -> (BL, 2E, L)
        enc_t = np.ascontiguousarray(
            np.asarray(enc_output[:, sl, :], dtype=np.float32).transpose(1, 2, 0)
        )
        enc8_t = enc_t.astype(ml_dtypes.float8_e4m3)
        encb_t = enc_t.astype(ml_dtypes.bfloat16)
        # (BL, DEC) -> [p, kt, b]
        dec_kpb = (
            np.ascontiguousarray(
                dec_hidden[sl].T.reshape(DT, 128, BL).transpose(1, 0, 2)
            )
            .astype(ml_dtypes.bfloat16)
        )
        in_maps.append(
            {
                "enc8_t": enc8_t,
                "encb_t": encb_t,
                "ones": np.ones((1, 128), dtype=np.float32),
                "dec_kpb": dec_kpb,
                "wenc8": wenc8,
                "w_dec": w_dec,
                "b_mat": b_mat,
                "v_mat": v_mat,
                "eye": eye,
            }
        )
    return in_maps


_NC_CACHE = None


def kernel(dec_hidden, enc_output, W_score, b_score, v):
    global _NC_CACHE
    if _NC_CACHE is None:
        _NC_CACHE = build_nc()
    nc = _NC_CACHE
    in_maps = shard_inputs(dec_hidden, enc_output, W_score, b_score, v)
    res = run_bass_kernel_spmd(nc, in_maps, list(range(N_CORES)))
    return np.concatenate([res.results[i]["att"] for i in range(N_CORES)], axis=0)


# revision 5
# speedup vs baseline: 2.0469x; 1.4433x over previous
"""Bahdanau additive-attention kernel for Trainium2, data-parallel over
batch across 8 NeuronCores.

Per batch b:
    energy  = tanh(dec_proj[b] + enc[b] @ W_enc + b_score)   # (L, DEC)
    scores  = energy @ v                                     # (L,)
    alpha   = softmax(scores)
    att[b]  = alpha @ enc[b]                                 # (2E,)

On-device layout (per core, 8 batches):
  - enc is staged host-side TWICE, both partition-major-tiled so DMA
    descriptors are 4-16KB runs: enc8_t (fp8e4m3, chunk-major) feeds the
    PE energy matmul in perf_mode=DoubleRow (two k-subtiles per
    instruction, measured ~2x over bf16/f32r at N=512); encb_t (bf16,
    full-L tiles) feeds the DVE attention reduce. W_enc is pre-scaled by
    128 on the host so its values sit mid-range in e4m3; the tanh
    activation rescales the psum by 1/128.
  - dec_proj preamble runs in bf16 (error negligible vs fp8 energy).
  - scores = v . energyT via PE matvec in bf16 over d-tiles.
  - softmax skips the max-subtraction: |scores| <= sum|v| = 32, safely
    inside the fp32 exp range. Raw scores broadcast to 128 partitions
    with a K=1 ones-matmul; Exp runs on the broadcast tile (bf16 out)
    with accum_out giving the replicated denominator per partition.
  - att^T accumulates via fused scalar_tensor_tensor on DVE:
    accum_out[e,1] = sum_l enc_bf16[e,l] * exp_scores[l], one full-L
    instruction per (batch, k-tile) to amortize DVE fixed overhead. The
    last batch runs per-chunk so only one chunk's reduce sits in the
    kernel tail.
  - startup is DMA-paced, so batch 0 chunk 0 consumes enc in half-tile
    arrival order with the dec_proj preamble matmuls behind it.
"""

import numpy as np
import ml_dtypes
from contextlib import ExitStack

import concourse.bass as bass
import concourse.tile as tile
from concourse import mybir
from concourse.bass_utils import run_bass_kernel_spmd
from concourse.vector_clock import ScopedClock, VectorClock

N_CORES = 8
B, L, DEC, ENC2 = 64, 1024, 1024, 2048
BL = B // N_CORES  # batches per core
KT = ENC2 // 128   # contraction tiles over e
KP = KT // 2       # DoubleRow pairs over e
KH = KT // 2       # k-tiles per half (SBUF tile granularity)
DT = DEC // 128    # d tiles
LC = 512           # l-chunk (one PSUM bank of f32)
NLC = L // LC
WSCALE = 128.0     # host-side W_enc scaling for fp8 range

F32 = mybir.dt.float32
F32R = mybir.dt.float32r
F8 = mybir.dt.float8e4
BF16 = mybir.dt.bfloat16
AF = mybir.ActivationFunctionType
ALU = mybir.AluOpType
PM = mybir.MatmulPerfMode


def _patch_tile_drain():
    """Workarounds for this container's walrus build.

    1. The Tile tail drain carries one sem wait per touched proc; walrus
       rejects >2 on the CTRL encoding. Split the waits onto single-wait
       SP nops (SP executes in order, so the drain then needs none).
    2. Any instruction with 2+ sem waits can fail codegen (the matmul
       LW encoding holds a single wait). Split multi-wait instructions:
       excess waits move onto same-engine InstNoOp carriers inserted
       just before; engine program order makes this equivalent.
    """
    if getattr(tile.TileContext, "_drain_patched", False):
        return

    def _drain_and_barrier(self, tick_clock, wait_clock):
        vec = list(tick_clock.global_clock)
        n = len(vec)
        for i in range(n):
            if vec[i] <= 0:
                continue
            part = [0] * n
            part[i] = vec[i]
            nop_inst = self.nc.sync.nop(nofuse=True)
            wait_clock.add_sem_waits(
                nop_inst.ins, ScopedClock({None: VectorClock(part)})
            )
        self.nc.sync.drain()
        self.nc.all_engine_barrier()
        assert self.sems is not None
        popped = self.nc._tile_sem_poison_stack.pop()
        assert popped is self._sem_poison
        self.nc.clear_and_free_semaphores(list(self.sems.allocated().values()))
        self.nc.all_engine_barrier()

    tile.TileContext._drain_and_barrier = _drain_and_barrier

    import bass_rust

    orig_lower = tile.TileContext._lower_ordered_insts

    def _lower_with_wait_split(self, ordered):
        for insts in ordered.values():
            expanded = []
            for inst in insts:
                si = inst.sync_info
                waits = list(si.on_wait) if si and si.on_wait else []
                if len(waits) > 1:
                    for w in waits[:-1]:
                        nop = mybir.InstNoOp(
                            name=self.nc.get_next_instruction_name(),
                            engine=inst.engine,
                            bass_nofuse=True,
                            sync_info=bass_rust.SyncInfo(on_wait=[w], on_update=[]),
                        )
                        self.nc.register_instruction(nop)
                        expanded.append(nop)
                    inst.sync_info = bass_rust.SyncInfo(
                        on_wait=[waits[-1]],
                        on_update=list(si.on_update) if si.on_update else [],
                    )
                expanded.append(inst)
            insts[:] = expanded
        return orig_lower(self, ordered)

    tile.TileContext._lower_ordered_insts = _lower_with_wait_split
    tile.TileContext._drain_patched = True


def build_nc():
    _patch_tile_drain()
    nc = bass.Bass()
    # partition-major tiled layouts (see shard_inputs)
    enc8_t = nc.declare_dram_parameter(
        "enc8_t", [BL, NLC, 2, 128, KH, LC], F8, isOutput=False
    )
    encb_t = nc.declare_dram_parameter(
        "encb_t", [BL, 2, 128, KH, L], BF16, isOutput=False
    )
    wenc8_d = nc.declare_dram_parameter(
        "wenc8", [128, KT, DEC], F8, isOutput=False
    )
    wd_d = nc.declare_dram_parameter(
        "w_dec", [128, DT * DT * 128], BF16, isOutput=False
    )
    dec_kpb = nc.declare_dram_parameter("dec_kpb", [128, DT, BL], BF16, isOutput=False)
    b_mat = nc.declare_dram_parameter("b_mat", [128, DT], F32, isOutput=False)
    v_mat = nc.declare_dram_parameter("v_mat", [128, DT], BF16, isOutput=False)
    eye = nc.declare_dram_parameter("eye", [128, 128], F32, isOutput=False)
    ones = nc.declare_dram_parameter("ones", [1, 128], F32R, isOutput=False)
    att = nc.declare_dram_parameter("att", [BL, ENC2], F32, isOutput=True)

    with tile.TileContext(nc) as tc, ExitStack() as ctx:
        singles = ctx.enter_context(tc.tile_pool(name="singles", bufs=1))
        smalls = ctx.enter_context(tc.tile_pool(name="smalls", bufs=2))
        enc8_pool = ctx.enter_context(tc.tile_pool(name="enc8", bufs=8))
        encb_pool = ctx.enter_context(tc.tile_pool(name="encb", bufs=6))
        energy_pool = ctx.enter_context(tc.tile_pool(name="energy", bufs=3))
        wexp_pool = ctx.enter_context(tc.tile_pool(name="wexp", bufs=2))
        prod_pool = ctx.enter_context(tc.tile_pool(name="prod", bufs=2))
        ep_ps = ctx.enter_context(tc.tile_pool(name="ep_ps", bufs=4, space="PSUM"))
        sc_ps = ctx.enter_context(tc.tile_pool(name="sc_ps", bufs=1, space="PSUM"))
        wb_ps = ctx.enter_context(tc.tile_pool(name="wb_ps", bufs=1, space="PSUM"))
        dec_ps = ctx.enter_context(tc.tile_pool(name="dec_ps", bufs=1, space="PSUM"))
        att_ps_pool = ctx.enter_context(
            tc.tile_pool(name="att_ps", bufs=1, space="PSUM")
        )

        # ---- persistent tiles -------------------------------------------
        wenc = singles.tile([128, KT, DEC], F8)  # W_enc*128, (e-tile, k) x d
        wd_all = singles.tile([128, DT * DT, 128], BF16)  # (dt, kk) tiles
        dec_sb = singles.tile([128, DT, BL], BF16)
        b_sb = singles.tile([128, DT], F32)
        v_sb = singles.tile([128, DT], BF16)
        eye_sb = singles.tile([128, 128], F32)
        bias_sb = singles.tile([128, DT, BL], F32)  # dec_proj + b_score
        att_all = singles.tile([128, KT * BL], F32)  # att^T cols = b*KT+k
        ones_sb = singles.tile([1, 128], F32R)

        def wd_tile(dt, kk):
            return wd_all[:, dt * DT + kk, :]

        def alloc_chunk8(nm):
            a = enc8_pool.tile([128, KH, LC], F8, tag="enc", name=f"{nm}a")
            bb = enc8_pool.tile([128, KH, LC], F8, tag="enc", name=f"{nm}b")
            return (a, bb)

        def load_chunk8(b, c, ch):
            for h in range(2):
                nc.sync.dma_start(out=ch[h], in_=enc8_t[b, c, h])

        def alloc_encb(nm):
            a = encb_pool.tile([128, KH, L], BF16, tag="encb", name=f"{nm}a")
            bb = encb_pool.tile([128, KH, L], BF16, tag="encb", name=f"{nm}b")
            return (a, bb)

        def load_encb(b, ch):
            for h in range(2):
                nc.sync.dma_start(out=ch[h], in_=encb_t[b, h])

        def enc_pair(ch, kp):
            """[128, 2, LC] DoubleRow rhs slice for pair (2kp, 2kp+1)."""
            k = 2 * kp
            t, kk = (ch[0], k) if k < KH else (ch[1], k - KH)
            return t[:, kk : kk + 2, :]

        def encb_sl(ch, k, lo=0, width=L):
            t, kk = (ch[0], k) if k < KH else (ch[1], k - KH)
            return t[:, kk, lo : lo + width]

        def w_pair(kp, dt):
            """[128, 2, 128] DoubleRow lhsT slice."""
            return wenc[:, 2 * kp : 2 * kp + 2, dt * 128 : (dt + 1) * 128]

        # ---- startup DMA, in data-arrival order -------------------------
        nc.sync.dma_start(out=dec_sb, in_=dec_kpb[:, :, :])
        nc.sync.dma_start(out=b_sb, in_=b_mat[:, :])
        enc00 = alloc_chunk8("enc00")
        nc.sync.dma_start(out=enc00[0], in_=enc8_t[0, 0, 0])
        nc.sync.dma_start(out=wenc[:, 0:4, :], in_=wenc8_d[:, 0:4, :])
        nc.sync.dma_start(out=enc00[1], in_=enc8_t[0, 0, 1])
        for q in range(1, 4):
            nc.sync.dma_start(
                out=wenc[:, 4 * q : 4 * q + 4, :], in_=wenc8_d[:, 4 * q : 4 * q + 4, :]
            )
        for q in range(4):
            nc.sync.dma_start(
                out=wd_all[:, 16 * q : 16 * q + 16, :],
                in_=wd_d[:, 16 * q * 128 : (16 * q + 16) * 128].rearrange(
                    "p (i m) -> p i m", m=128
                ),
            )
        nc.sync.dma_start(out=v_sb, in_=v_mat[:, :])
        nc.sync.dma_start(out=eye_sb, in_=eye[:, :])
        nc.sync.dma_start(out=ones_sb, in_=ones[:, :])

        # ---- chunk (0,0): consume pairs in half-arrival order -----------
        ps00 = {
            dt: ep_ps.tile([128, LC], F32, tag="ep", name=f"ps00_{dt}")
            for dt in range(4)
        }
        for kp in range(KP):
            for dt in range(4):
                nc.tensor.matmul(
                    ps00[dt],
                    lhsT=w_pair(kp, dt),
                    rhs=enc_pair(enc00, kp),
                    start=(kp == 0),
                    stop=(kp == KP - 1),
                    perf_mode=PM.DoubleRow,
                )

        # dec_proj preamble (bf16): runs behind chunk00's matmuls
        dpsum = dec_ps.tile([128, BL], F32)

        def emit_pre_group(dt):
            for kk in range(DT):
                nc.tensor.matmul(
                    dpsum,
                    lhsT=wd_tile(dt, kk),
                    rhs=dec_sb[:, kk, :],
                    start=(kk == 0),
                    stop=(kk == DT - 1),
                )
            nc.vector.tensor_scalar_add(
                out=bias_sb[:, dt, :], in0=dpsum, scalar1=b_sb[:, dt : dt + 1]
            )

        for dt in range(DT):
            emit_pre_group(dt)

        encb0 = alloc_encb("encb0")
        load_encb(0, encb0)

        def tanh_and_score(ps, sc, b, dt, lo=0, width=LC):
            energy = energy_pool.tile([128, LC], BF16, tag="energy")
            nc.scalar.activation(
                out=energy[:, 0:width],
                in_=ps,
                func=AF.Tanh,
                bias=bias_sb[:, dt, b : b + 1],
                scale=1.0 / WSCALE,
            )
            nc.tensor.matmul(
                sc[:, lo : lo + width],
                lhsT=v_sb[:, dt : dt + 1],
                rhs=energy[:, 0:width],
                start=(dt == 0),
                stop=(dt == DT - 1),
            )

        sc00 = sc_ps.tile([1, LC], F32, tag="sc")
        for dt in range(4):
            tanh_and_score(ps00[dt], sc00, 0, dt)
        for dt in range(4, DT):
            ps = ep_ps.tile([128, LC], F32, tag="ep", name=f"ps00b_{dt}")
            for kp in range(KP):
                nc.tensor.matmul(
                    ps,
                    lhsT=w_pair(kp, dt),
                    rhs=enc_pair(enc00, kp),
                    start=(kp == 0),
                    stop=(kp == KP - 1),
                    perf_mode=PM.DoubleRow,
                )
            tanh_and_score(ps, sc00, 0, dt)

        def kmajor_chunk(b, c, enc_tile):
            """Compute one chunk's scores consuming enc pairs in DMA
            arrival order: dt 0-3 accumulate pair-major across 4 psum
            groups, then dt 4-7 run dt-major at full speed."""
            ps = {
                dt: ep_ps.tile([128, LC], F32, tag="ep", name=f"km_{b}_{c}_{dt}")
                for dt in range(4)
            }
            for kp in range(KP):
                for dt in range(4):
                    nc.tensor.matmul(
                        ps[dt],
                        lhsT=w_pair(kp, dt),
                        rhs=enc_pair(enc_tile, kp),
                        start=(kp == 0),
                        stop=(kp == KP - 1),
                        perf_mode=PM.DoubleRow,
                    )
            sc = sc_ps.tile([1, LC], F32, tag="sc", name=f"km_sc_{b}_{c}")
            for dt in range(4):
                tanh_and_score(ps[dt], sc, b, dt)
            for dt in range(4, DT):
                p2 = ep_ps.tile([128, LC], F32, tag="ep", name=f"km2_{b}_{c}_{dt}")
                for kp in range(KP):
                    nc.tensor.matmul(
                        p2,
                        lhsT=w_pair(kp, dt),
                        rhs=enc_pair(enc_tile, kp),
                        start=(kp == 0),
                        stop=(kp == KP - 1),
                        perf_mode=PM.DoubleRow,
                    )
                tanh_and_score(p2, sc, b, dt)
            return sc

        def dtmajor_chunk(b, c, enc_tile):
            sc = sc_ps.tile([1, LC], F32, tag="sc", name=f"dm_sc_{b}_{c}")
            for dt in range(DT):
                ps = ep_ps.tile([128, LC], F32, tag="ep", name=f"dm_{b}_{c}_{dt}")
                for kp in range(KP):
                    nc.tensor.matmul(
                        ps,
                        lhsT=w_pair(kp, dt),
                        rhs=enc_pair(enc_tile, kp),
                        start=(kp == 0),
                        stop=(kp == KP - 1),
                        perf_mode=PM.DoubleRow,
                    )
                tanh_and_score(ps, sc, b, dt)
            return sc

        def score_to_wexp(sc, wexp, b, c):
            """Exp the raw chunk scores into the batch's broadcast weight
            tile; returns the per-partition denominator contribution."""
            s_sb = smalls.tile([1, LC], F32R, tag="ssb", name=f"ssb_{b}_{c}")
            nc.scalar.copy(out=s_sb, in_=sc)
            wb = wb_ps.tile([128, LC], F32, tag="wb", name=f"wb_{b}_{c}")
            nc.tensor.matmul(wb, lhsT=ones_sb, rhs=s_sb, start=True, stop=True)
            den_c = smalls.tile([128, 1], F32, tag=f"den{c}", name=f"den_{b}_{c}")
            nc.scalar.activation(
                out=wexp[:, c * LC : (c + 1) * LC],
                in_=wb,
                func=AF.Exp,
                bias=0.0,
                scale=1.0,
                accum_out=den_c,
            )
            return den_c

        def batch_att(b, encb_tile, wexp):
            """Fused weighted reduce over the full L per k-tile."""
            for k in range(KT):
                col = b * KT + k
                prod = prod_pool.tile([128, L], BF16, tag="prod", name=f"pr_{b}_{k}")
                nc.vector.scalar_tensor_tensor(
                    out=prod,
                    in0=encb_sl(encb_tile, k),
                    scalar=1.0,
                    in1=wexp,
                    op0=ALU.mult,
                    op1=ALU.mult,
                    accum_out=att_all[:, col : col + 1],
                )

        def chunk_att(b, c, encb_tile, wexp):
            """Per-chunk variant (used for the last batch to keep the
            kernel tail to one chunk's reduce)."""
            atmp = None
            if c > 0:
                atmp = smalls.tile([128, KT], F32, tag="atmp", name=f"atmp_{b}_{c}")
            for k in range(KT):
                col = b * KT + k
                prod = prod_pool.tile(
                    [128, L], BF16, tag="prod", name=f"prc_{b}_{c}_{k}"
                )
                dst = att_all[:, col : col + 1] if c == 0 else atmp[:, k : k + 1]
                nc.vector.scalar_tensor_tensor(
                    out=prod[:, 0:LC],
                    in0=encb_sl(encb_tile, k, c * LC, LC),
                    scalar=1.0,
                    in1=wexp[:, c * LC : (c + 1) * LC],
                    op0=ALU.mult,
                    op1=ALU.mult,
                    accum_out=dst,
                )
            if c > 0:
                cols = slice(b * KT, (b + 1) * KT)
                nc.vector.tensor_add(
                    out=att_all[:, cols], in0=att_all[:, cols], in1=atmp
                )

        def batch_epilogue(b, dens):
            """Normalize, transpose, and store one batch's attention row."""
            rden = smalls.tile([128, 1], F32, tag="rden")
            nc.vector.tensor_add(out=rden, in0=dens[0], in1=dens[1])
            for extra in dens[2:]:
                nc.vector.tensor_add(out=rden, in0=rden, in1=extra)
            nc.vector.reciprocal(out=rden, in_=rden)
            cols = slice(b * KT, (b + 1) * KT)
            nc.vector.tensor_scalar_mul(att_all[:, cols], att_all[:, cols], rden)
            att_bt = att_ps_pool.tile([KT, 128], F32, tag="abt")
            nc.tensor.transpose(att_bt, att_all[:, cols], eye_sb)
            att_sb = smalls.tile([KT, 128], F32, tag="asb")
            nc.vector.tensor_copy(out=att_sb, in_=att_bt)
            nc.sync.dma_start(
                out=att[b].rearrange("(k p) -> k p", p=128), in_=att_sb
            )

        # ---- main loop (chunk (0,0) scores already computed above) ------
        wexp_b = wexp_pool.tile([128, L], BF16, tag="wexp", name="wexp_0")
        dens = [score_to_wexp(sc00, wexp_b, 0, 0)]
        encb_b = encb0
        encb_next = None
        for b in range(BL):
            last_b = b == BL - 1
            for c in range(NLC):
                if (b, c) == (0, 0):
                    continue
                enc_tile = alloc_chunk8(f"enc_{b}_{c}")
                load_chunk8(b, c, enc_tile)
                if c == 0:
                    wexp_b = wexp_pool.tile(
                        [128, L], BF16, tag="wexp", name=f"wexp_{b}"
                    )
                    if b > 0:
                        encb_b = encb_next
                elif b + 1 < BL:
                    # prefetch next batch's bf16 copy a chunk early
                    encb_next = alloc_encb(f"encb_{b + 1}")
                    load_encb(b + 1, encb_next)
                if (b, c) in ((0, 1), (1, 0), (1, 1)):
                    sc = kmajor_chunk(b, c, enc_tile)
                else:
                    sc = dtmajor_chunk(b, c, enc_tile)
                dens.append(score_to_wexp(sc, wexp_b, b, c))
                if last_b:
                    chunk_att(b, c, encb_b, wexp_b)
            if not last_b:
                batch_att(b, encb_b, wexp_b)
            batch_epilogue(b, dens)
            dens = []

    return nc


def shard_inputs(dec_hidden, enc_output, W_score, b_score, v):
    """Full inputs -> per-core input maps (host-side layout staging)."""
    dec_hidden = np.ascontiguousarray(dec_hidden, dtype=np.float32)
    W_score = np.asarray(W_score, dtype=np.float32)
    # W_dec tiled partition-major: [p, dt, kk, m] with value W[kk*128+p, dt*128+m]
    wd_t = np.ascontiguousarray(
        W_score[:DEC]
        .reshape(DT, 128, DT, 128)
        .transpose(1, 2, 0, 3)
        .reshape(128, DT * DT * 128)
    ).astype(ml_dtypes.bfloat16)
    # W_enc tiled partition-major: [p, k, d]
    wenc8 = np.ascontiguousarray(
        (W_score[DEC:] * WSCALE).reshape(KT, 128, DEC).transpose(1, 0, 2)
    ).astype(ml_dtypes.float8_e4m3)
    b_mat = np.ascontiguousarray(
        np.asarray(b_score, dtype=np.float32).reshape(DT, 128).T
    )
    v_mat = (
        np.ascontiguousarray(np.asarray(v, dtype=np.float32).reshape(DT, 128).T)
        .astype(ml_dtypes.bfloat16)
    )
    eye = np.eye(128, dtype=np.float32)

    in_maps = []
    for core in range(N_CORES):
        sl = slice(core * BL, (core + 1) * BL)
        # (L, BL, 2E) -> (BL, 2E, L)
        enc_t = np.ascontiguousarray(
            np.asarray(enc_output[:, sl, :], dtype=np.float32).transpose(1, 2, 0)
        )
        # fp8 chunk-major partition-tiled: [b, c, half, p, k, l]
        enc8_t = np.ascontiguousarray(
            enc_t.reshape(BL, 2, KH, 128, NLC, LC).transpose(0, 4, 1, 3, 2, 5)
        ).astype(ml_dtypes.float8_e4m3)
        # bf16 full-L partition-tiled: [b, half, p, k, l]
        encb_t = np.ascontiguousarray(
            enc_t.reshape(BL, 2, KH, 128, L).transpose(0, 1, 3, 2, 4)
        ).astype(ml_dtypes.bfloat16)
        # (BL, DEC) -> [p, kt, b]
        dec_kpb = (
            np.ascontiguousarray(
                dec_hidden[sl].T.reshape(DT, 128, BL).transpose(1, 0, 2)
            )
            .astype(ml_dtypes.bfloat16)
        )
        in_maps.append(
            {
                "enc8_t": enc8_t,
                "encb_t": encb_t,
                "ones": np.ones((1, 128), dtype=np.float32),
                "dec_kpb": dec_kpb,
                "wenc8": wenc8,
                "w_dec": wd_t,
                "b_mat": b_mat,
                "v_mat": v_mat,
                "eye": eye,
            }
        )
    return in_maps


_NC_CACHE = None


def kernel(dec_hidden, enc_output, W_score, b_score, v):
    global _NC_CACHE
    if _NC_CACHE is None:
        _NC_CACHE = build_nc()
    nc = _NC_CACHE
    in_maps = shard_inputs(dec_hidden, enc_output, W_score, b_score, v)
    res = run_bass_kernel_spmd(nc, in_maps, list(range(N_CORES)))
    return np.concatenate([res.results[i]["att"] for i in range(N_CORES)], axis=0)


# revision 8
# speedup vs baseline: 2.0867x; 1.0194x over previous
"""Bahdanau additive-attention kernel for Trainium2, data-parallel over
batch across 8 NeuronCores.

Per batch b:
    energy  = tanh(dec_proj[b] + enc[b] @ W_enc + b_score)   # (L, DEC)
    scores  = energy @ v                                     # (L,)
    alpha   = softmax(scores)
    att[b]  = alpha @ enc[b]                                 # (2E,)

On-device layout (per core, 8 batches):
  - enc is staged host-side TWICE, both partition-major-tiled so DMA
    descriptors are 4-16KB runs: enc8_t (fp8e4m3, chunk-major) feeds the
    PE energy matmul in perf_mode=DoubleRow (two k-subtiles per
    instruction, measured ~2x over bf16/f32r at N=512); encb_t (bf16,
    full-L tiles) feeds the DVE attention reduce. W_enc is pre-scaled by
    128 on the host so its values sit mid-range in e4m3; the tanh
    activation rescales the psum by 1/128.
  - dec_proj preamble runs in bf16 (error negligible vs fp8 energy).
  - scores = v . energyT via PE matvec in bf16 over d-tiles.
  - softmax skips the max-subtraction: |scores| <= sum|v| = 32, safely
    inside the fp32 exp range. Raw scores broadcast to 128 partitions
    with a K=1 ones-matmul; Exp runs on the broadcast tile (bf16 out)
    with accum_out giving the replicated denominator per partition.
  - att^T accumulates via fused scalar_tensor_tensor on DVE:
    accum_out[e,1] = sum_l enc_bf16[e,l] * exp_scores[l], one full-L
    instruction per (batch, k-tile) to amortize DVE fixed overhead. The
    last batch runs per-chunk so only one chunk's reduce sits in the
    kernel tail.
  - startup is DMA-paced, so batch 0 chunk 0 consumes enc in half-tile
    arrival order with the dec_proj preamble matmuls behind it.
"""

import numpy as np
import ml_dtypes
from contextlib import ExitStack

import concourse.bass as bass
import concourse.tile as tile
from concourse import mybir
from concourse.bass_utils import run_bass_kernel_spmd
from concourse.vector_clock import ScopedClock, VectorClock

N_CORES = 8
B, L, DEC, ENC2 = 64, 1024, 1024, 2048
BL = B // N_CORES  # batches per core
KT = ENC2 // 128   # contraction tiles over e
KP = KT // 2       # DoubleRow pairs over e
KH = KT // 2       # k-tiles per half (SBUF tile granularity)
DT = DEC // 128    # d tiles
LC = 512           # l-chunk (one PSUM bank of f32)
NLC = L // LC
WSCALE = 128.0     # host-side W_enc scaling for fp8 range

F32 = mybir.dt.float32
F32R = mybir.dt.float32r
F8 = mybir.dt.float8e4
BF16 = mybir.dt.bfloat16
AF = mybir.ActivationFunctionType
ALU = mybir.AluOpType
PM = mybir.MatmulPerfMode


def _patch_tile_drain():
    """Workarounds for this container's walrus build.

    1. The Tile tail drain carries one sem wait per touched proc; walrus
       rejects >2 on the CTRL encoding. Split the waits onto single-wait
       SP nops (SP executes in order, so the drain then needs none).
    2. Any instruction with 2+ sem waits can fail codegen (the matmul
       LW encoding holds a single wait). Split multi-wait instructions:
       excess waits move onto same-engine InstNoOp carriers inserted
       just before; engine program order makes this equivalent.
    """
    if getattr(tile.TileContext, "_drain_patched", False):
        return

    def _drain_and_barrier(self, tick_clock, wait_clock):
        vec = list(tick_clock.global_clock)
        n = len(vec)
        for i in range(n):
            if vec[i] <= 0:
                continue
            part = [0] * n
            part[i] = vec[i]
            nop_inst = self.nc.sync.nop(nofuse=True)
            wait_clock.add_sem_waits(
                nop_inst.ins, ScopedClock({None: VectorClock(part)})
            )
        self.nc.sync.drain()
        self.nc.all_engine_barrier()
        assert self.sems is not None
        popped = self.nc._tile_sem_poison_stack.pop()
        assert popped is self._sem_poison
        self.nc.clear_and_free_semaphores(list(self.sems.allocated().values()))
        self.nc.all_engine_barrier()

    tile.TileContext._drain_and_barrier = _drain_and_barrier

    import bass_rust

    orig_lower = tile.TileContext._lower_ordered_insts

    def _lower_with_wait_split(self, ordered):
        for insts in ordered.values():
            expanded = []
            for inst in insts:
                si = inst.sync_info
                waits = list(si.on_wait) if si and si.on_wait else []
                if len(waits) > 1:
                    for w in waits[:-1]:
                        nop = mybir.InstNoOp(
                            name=self.nc.get_next_instruction_name(),
                            engine=inst.engine,
                            bass_nofuse=True,
                            sync_info=bass_rust.SyncInfo(on_wait=[w], on_update=[]),
                        )
                        self.nc.register_instruction(nop)
                        expanded.append(nop)
                    inst.sync_info = bass_rust.SyncInfo(
                        on_wait=[waits[-1]],
                        on_update=list(si.on_update) if si.on_update else [],
                    )
                expanded.append(inst)
            insts[:] = expanded
        return orig_lower(self, ordered)

    tile.TileContext._lower_ordered_insts = _lower_with_wait_split
    tile.TileContext._drain_patched = True


def build_nc():
    _patch_tile_drain()
    nc = bass.Bass()
    # partition-major tiled layouts (see shard_inputs)
    enc8_t = nc.declare_dram_parameter(
        "enc8_t", [BL, NLC, 2, 128, KH, LC], F8, isOutput=False
    )
    encb_t = nc.declare_dram_parameter(
        "encb_t", [BL, 2, 128, KH, L], BF16, isOutput=False
    )
    wenc8_d = nc.declare_dram_parameter(
        "wenc8", [128, KT, DEC], F8, isOutput=False
    )
    wd_d = nc.declare_dram_parameter(
        "w_dec", [128, DT * DT * 128], BF16, isOutput=False
    )
    dec_kpb = nc.declare_dram_parameter("dec_kpb", [128, DT, BL], BF16, isOutput=False)
    b_mat = nc.declare_dram_parameter("b_mat", [128, DT], F32, isOutput=False)
    v_mat = nc.declare_dram_parameter("v_mat", [128, DT], BF16, isOutput=False)
    eye = nc.declare_dram_parameter("eye", [128, 128], F32, isOutput=False)
    ones = nc.declare_dram_parameter("ones", [1, 128], F32R, isOutput=False)
    att = nc.declare_dram_parameter("att", [BL, ENC2], F32, isOutput=True)

    with tile.TileContext(nc) as tc, ExitStack() as ctx:
        singles = ctx.enter_context(tc.tile_pool(name="singles", bufs=1))
        smalls = ctx.enter_context(tc.tile_pool(name="smalls", bufs=2))
        enc8_pool = ctx.enter_context(tc.tile_pool(name="enc8", bufs=8))
        encb_pool = ctx.enter_context(tc.tile_pool(name="encb", bufs=6))
        energy_pool = ctx.enter_context(tc.tile_pool(name="energy", bufs=3))
        wexp_pool = ctx.enter_context(tc.tile_pool(name="wexp", bufs=2))
        prod_pool = ctx.enter_context(tc.tile_pool(name="prod", bufs=2))
        ep_ps = ctx.enter_context(tc.tile_pool(name="ep_ps", bufs=4, space="PSUM"))
        sc_ps = ctx.enter_context(tc.tile_pool(name="sc_ps", bufs=1, space="PSUM"))
        wb_ps = ctx.enter_context(tc.tile_pool(name="wb_ps", bufs=1, space="PSUM"))
        dec_ps = ctx.enter_context(tc.tile_pool(name="dec_ps", bufs=1, space="PSUM"))
        att_ps_pool = ctx.enter_context(
            tc.tile_pool(name="att_ps", bufs=1, space="PSUM")
        )

        # ---- persistent tiles -------------------------------------------
        wenc = singles.tile([128, KT, DEC], F8)  # W_enc*128, (e-tile, k) x d
        wd_all = singles.tile([128, DT * DT, 128], BF16)  # (dt, kk) tiles
        dec_sb = singles.tile([128, DT, BL], BF16)
        b_sb = singles.tile([128, DT], F32)
        v_sb = singles.tile([128, DT], BF16)
        eye_sb = singles.tile([128, 128], F32)
        bias_sb = singles.tile([128, DT, BL], F32)  # dec_proj + b_score
        att_all = singles.tile([128, KT * BL], F32)  # att^T cols = b*KT+k
        ones_sb = singles.tile([1, 128], F32R)

        def wd_tile(dt, kk):
            return wd_all[:, dt * DT + kk, :]

        def alloc_chunk8(nm):
            a = enc8_pool.tile([128, KH, LC], F8, tag="enc", name=f"{nm}a")
            bb = enc8_pool.tile([128, KH, LC], F8, tag="enc", name=f"{nm}b")
            return (a, bb)

        def load_chunk8(b, c, ch):
            for h in range(2):
                nc.sync.dma_start(out=ch[h], in_=enc8_t[b, c, h])

        def alloc_encb(nm):
            a = encb_pool.tile([128, KH, L], BF16, tag="encb", name=f"{nm}a")
            bb = encb_pool.tile([128, KH, L], BF16, tag="encb", name=f"{nm}b")
            return (a, bb)

        def load_encb(b, ch):
            for h in range(2):
                nc.sync.dma_start(out=ch[h], in_=encb_t[b, h])

        def enc_pair(ch, kp):
            """[128, 2, LC] DoubleRow rhs slice for pair (2kp, 2kp+1)."""
            k = 2 * kp
            t, kk = (ch[0], k) if k < KH else (ch[1], k - KH)
            return t[:, kk : kk + 2, :]

        def encb_sl(ch, k, lo=0, width=L):
            t, kk = (ch[0], k) if k < KH else (ch[1], k - KH)
            return t[:, kk, lo : lo + width]

        def w_pair(kp, dt):
            """[128, 2, 128] DoubleRow lhsT slice."""
            return wenc[:, 2 * kp : 2 * kp + 2, dt * 128 : (dt + 1) * 128]

        # ---- startup DMA, in data-arrival order -------------------------
        nc.sync.dma_start(out=dec_sb, in_=dec_kpb[:, :, :])
        nc.sync.dma_start(out=b_sb, in_=b_mat[:, :])
        enc00 = alloc_chunk8("enc00")
        nc.sync.dma_start(out=enc00[0], in_=enc8_t[0, 0, 0])
        nc.sync.dma_start(out=wenc[:, 0:4, :], in_=wenc8_d[:, 0:4, :])
        nc.sync.dma_start(out=enc00[1], in_=enc8_t[0, 0, 1])
        for q in range(1, 4):
            nc.sync.dma_start(
                out=wenc[:, 4 * q : 4 * q + 4, :], in_=wenc8_d[:, 4 * q : 4 * q + 4, :]
            )
        for q in range(4):
            nc.sync.dma_start(
                out=wd_all[:, 16 * q : 16 * q + 16, :],
                in_=wd_d[:, 16 * q * 128 : (16 * q + 16) * 128].rearrange(
                    "p (i m) -> p i m", m=128
                ),
            )
        nc.sync.dma_start(out=v_sb, in_=v_mat[:, :])
        nc.sync.dma_start(out=eye_sb, in_=eye[:, :])
        nc.sync.dma_start(out=ones_sb, in_=ones[:, :])

        # ---- chunk (0,0): consume pairs in half-arrival order -----------
        ps00 = {
            dt: ep_ps.tile([128, LC], F32, tag="ep", name=f"ps00_{dt}")
            for dt in range(4)
        }
        for kp in range(KP):
            for dt in range(4):
                nc.tensor.matmul(
                    ps00[dt],
                    lhsT=w_pair(kp, dt),
                    rhs=enc_pair(enc00, kp),
                    start=(kp == 0),
                    stop=(kp == KP - 1),
                    perf_mode=PM.DoubleRow,
                )

        # dec_proj preamble (bf16): runs behind chunk00's matmuls
        dpsum = dec_ps.tile([128, BL], F32)

        def emit_pre_group(dt):
            for kk in range(DT):
                nc.tensor.matmul(
                    dpsum,
                    lhsT=wd_tile(dt, kk),
                    rhs=dec_sb[:, kk, :],
                    start=(kk == 0),
                    stop=(kk == DT - 1),
                )
            nc.vector.tensor_scalar_add(
                out=bias_sb[:, dt, :], in0=dpsum, scalar1=b_sb[:, dt : dt + 1]
            )

        for dt in range(DT):
            emit_pre_group(dt)

        def tanh_and_score(ps, sc, b, dt, lo=0, width=LC):
            energy = energy_pool.tile([128, LC], BF16, tag="energy")
            nc.scalar.activation(
                out=energy[:, 0:width],
                in_=ps,
                func=AF.Tanh,
                bias=bias_sb[:, dt, b : b + 1],
                scale=1.0 / WSCALE,
            )
            nc.tensor.matmul(
                sc[:, lo : lo + width],
                lhsT=v_sb[:, dt : dt + 1],
                rhs=energy[:, 0:width],
                start=(dt == 0),
                stop=(dt == DT - 1),
            )

        sc00 = sc_ps.tile([1, LC], F32, tag="sc")
        for dt in range(4):
            tanh_and_score(ps00[dt], sc00, 0, dt)
        for dt in range(4, DT):
            ps = ep_ps.tile([128, LC], F32, tag="ep", name=f"ps00b_{dt}")
            for kp in range(KP):
                nc.tensor.matmul(
                    ps,
                    lhsT=w_pair(kp, dt),
                    rhs=enc_pair(enc00, kp),
                    start=(kp == 0),
                    stop=(kp == KP - 1),
                    perf_mode=PM.DoubleRow,
                )
            tanh_and_score(ps, sc00, 0, dt)

        def kmajor_chunk(b, c, enc_tile):
            """Compute one chunk's scores consuming enc pairs in DMA
            arrival order: dt 0-3 accumulate pair-major across 4 psum
            groups, then dt 4-7 run dt-major at full speed."""
            ps = {
                dt: ep_ps.tile([128, LC], F32, tag="ep", name=f"km_{b}_{c}_{dt}")
                for dt in range(4)
            }
            for kp in range(KP):
                for dt in range(4):
                    nc.tensor.matmul(
                        ps[dt],
                        lhsT=w_pair(kp, dt),
                        rhs=enc_pair(enc_tile, kp),
                        start=(kp == 0),
                        stop=(kp == KP - 1),
                        perf_mode=PM.DoubleRow,
                    )
            sc = sc_ps.tile([1, LC], F32, tag="sc", name=f"km_sc_{b}_{c}")
            for dt in range(4):
                tanh_and_score(ps[dt], sc, b, dt)
            for dt in range(4, DT):
                p2 = ep_ps.tile([128, LC], F32, tag="ep", name=f"km2_{b}_{c}_{dt}")
                for kp in range(KP):
                    nc.tensor.matmul(
                        p2,
                        lhsT=w_pair(kp, dt),
                        rhs=enc_pair(enc_tile, kp),
                        start=(kp == 0),
                        stop=(kp == KP - 1),
                        perf_mode=PM.DoubleRow,
                    )
                tanh_and_score(p2, sc, b, dt)
            return sc

        def dtmajor_chunk(b, c, enc_tile):
            sc = sc_ps.tile([1, LC], F32, tag="sc", name=f"dm_sc_{b}_{c}")
            for dt in range(DT):
                ps = ep_ps.tile([128, LC], F32, tag="ep", name=f"dm_{b}_{c}_{dt}")
                for kp in range(KP):
                    nc.tensor.matmul(
                        ps,
                        lhsT=w_pair(kp, dt),
                        rhs=enc_pair(enc_tile, kp),
                        start=(kp == 0),
                        stop=(kp == KP - 1),
                        perf_mode=PM.DoubleRow,
                    )
                tanh_and_score(ps, sc, b, dt)
            return sc

        def score_to_wexp(sc, wexp, b, c):
            """Exp the raw chunk scores into the batch's broadcast weight
            tile; returns the per-partition denominator contribution."""
            s_sb = smalls.tile([1, LC], F32R, tag="ssb", name=f"ssb_{b}_{c}")
            nc.scalar.copy(out=s_sb, in_=sc)
            wb = wb_ps.tile([128, LC], F32, tag="wb", name=f"wb_{b}_{c}")
            nc.tensor.matmul(wb, lhsT=ones_sb, rhs=s_sb, start=True, stop=True)
            den_c = smalls.tile([128, 1], F32, tag=f"den{c}", name=f"den_{b}_{c}")
            nc.scalar.activation(
                out=wexp[:, c * LC : (c + 1) * LC],
                in_=wb,
                func=AF.Exp,
                bias=0.0,
                scale=1.0,
                accum_out=den_c,
            )
            return den_c

        def batch_att(b, encb_tile, wexp):
            """Fused weighted reduce over the full L per k-tile."""
            for k in range(KT):
                col = b * KT + k
                prod = prod_pool.tile([128, L], BF16, tag="prod", name=f"pr_{b}_{k}")
                nc.vector.scalar_tensor_tensor(
                    out=prod,
                    in0=encb_sl(encb_tile, k),
                    scalar=1.0,
                    in1=wexp,
                    op0=ALU.mult,
                    op1=ALU.mult,
                    accum_out=att_all[:, col : col + 1],
                )

        def chunk_att(b, c, encb_tile, wexp):
            """Per-chunk variant (used for the last batch to keep the
            kernel tail to one chunk's reduce)."""
            atmp = None
            if c > 0:
                atmp = smalls.tile([128, KT], F32, tag="atmp", name=f"atmp_{b}_{c}")
            for k in range(KT):
                col = b * KT + k
                prod = prod_pool.tile(
                    [128, L], BF16, tag="prod", name=f"prc_{b}_{c}_{k}"
                )
                dst = att_all[:, col : col + 1] if c == 0 else atmp[:, k : k + 1]
                nc.vector.scalar_tensor_tensor(
                    out=prod[:, 0:LC],
                    in0=encb_sl(encb_tile, k, c * LC, LC),
                    scalar=1.0,
                    in1=wexp[:, c * LC : (c + 1) * LC],
                    op0=ALU.mult,
                    op1=ALU.mult,
                    accum_out=dst,
                )
            if c > 0:
                cols = slice(b * KT, (b + 1) * KT)
                nc.vector.tensor_add(
                    out=att_all[:, cols], in0=att_all[:, cols], in1=atmp
                )

        def chunk_att_tail(b, c, encb_tile, wexp):
            """Kernel-tail variant: split the reduce between DVE (fused
            STT, k 0-7) and ACT (accum-copy over DVE pair-products,
            k 8-15) so the exposed tail is ~2/3 shorter."""
            atmp = smalls.tile([128, KT], F32, tag="atmp", name=f"atmpt_{b}_{c}")
            w_sl = wexp[:, c * LC : (c + 1) * LC]
            w_pairbc = bass.AP(
                tensor=w_sl.tensor,
                offset=w_sl.offset,
                ap=[w_sl.ap[0], [0, 2], w_sl.ap[1]],
            )
            # DVE pair-products for ACT's half first, so ACT starts early
            pprods = []
            for kp in range(KT // 4, KT // 2):
                k = 2 * kp
                t, kk = (encb_tile[0], k) if k < KH else (encb_tile[1], k - KH)
                prod = prod_pool.tile(
                    [128, 2, LC], BF16, tag="prodp", name=f"prp_{b}_{c}_{kp}"
                )
                nc.vector.tensor_mul(
                    out=prod,
                    in0=t[:, kk : kk + 2, c * LC : (c + 1) * LC],
                    in1=w_pairbc,
                )
                pprods.append((k, prod))
            scr = smalls.tile([128, LC], BF16, tag="ascr", name=f"ascr_{b}_{c}")
            for k, prod in pprods:
                for i in range(2):
                    nc.scalar.activation(
                        out=scr,
                        in_=prod[:, i, :],
                        func=AF.Copy,
                        bias=0.0,
                        scale=1.0,
                        accum_out=atmp[:, k + i : k + i + 1],
                    )
            for k in range(KT // 2):
                prod = prod_pool.tile(
                    [128, L], BF16, tag="prod", name=f"prt_{b}_{c}_{k}"
                )
                nc.vector.scalar_tensor_tensor(
                    out=prod[:, 0:LC],
                    in0=encb_sl(encb_tile, k, c * LC, LC),
                    scalar=1.0,
                    in1=w_sl,
                    op0=ALU.mult,
                    op1=ALU.mult,
                    accum_out=atmp[:, k : k + 1],
                )
            cols = slice(b * KT, (b + 1) * KT)
            nc.vector.tensor_add(
                out=att_all[:, cols], in0=att_all[:, cols], in1=atmp
            )

        def batch_epilogue(b, dens):
            """Normalize, transpose, and store one batch's attention row."""
            rden = smalls.tile([128, 1], F32, tag="rden")
            nc.vector.tensor_add(out=rden, in0=dens[0], in1=dens[1])
            for extra in dens[2:]:
                nc.vector.tensor_add(out=rden, in0=rden, in1=extra)
            nc.vector.reciprocal(out=rden, in_=rden)
            cols = slice(b * KT, (b + 1) * KT)
            nc.vector.tensor_scalar_mul(att_all[:, cols], att_all[:, cols], rden)
            att_bt = att_ps_pool.tile([KT, 128], F32, tag="abt")
            nc.tensor.transpose(att_bt, att_all[:, cols], eye_sb)
            att_sb = smalls.tile([KT, 128], F32, tag="asb")
            nc.vector.tensor_copy(out=att_sb, in_=att_bt)
            nc.sync.dma_start(
                out=att[b].rearrange("(k p) -> k p", p=128), in_=att_sb
            )

        # ---- main loop (chunk (0,0) scores already computed above) ------
        wexp_b = wexp_pool.tile([128, L], BF16, tag="wexp", name="wexp_0")
        dens = [score_to_wexp(sc00, wexp_b, 0, 0)]
        encb_b = None
        encb_next = None
        for b in range(BL):
            last_b = b == BL - 1
            for c in range(NLC):
                if (b, c) == (0, 0):
                    continue
                enc_tile = alloc_chunk8(f"enc_{b}_{c}")
                load_chunk8(b, c, enc_tile)
                if c == 0:
                    wexp_b = wexp_pool.tile(
                        [128, L], BF16, tag="wexp", name=f"wexp_{b}"
                    )
                    if b == 1:
                        # batch 1's bf16 copy loads behind chunk (1,0)
                        encb_next = alloc_encb("encb_1")
                        load_encb(1, encb_next)
                    encb_b = encb_next
                else:
                    if b == 0:
                        # batch 0's bf16 copy loads behind chunk (0,1)
                        encb_b = alloc_encb("encb_0")
                        load_encb(0, encb_b)
                    if 1 <= b < BL - 1:
                        # prefetch next batch's bf16 copy a chunk early
                        encb_next = alloc_encb(f"encb_{b + 1}")
                        load_encb(b + 1, encb_next)
                if (b, c) in ((0, 1), (1, 0), (1, 1)):
                    sc = kmajor_chunk(b, c, enc_tile)
                else:
                    sc = dtmajor_chunk(b, c, enc_tile)
                dens.append(score_to_wexp(sc, wexp_b, b, c))
                if last_b:
                    if c == 0:
                        chunk_att(b, c, encb_b, wexp_b)
                    else:
                        chunk_att_tail(b, c, encb_b, wexp_b)
            if not last_b:
                batch_att(b, encb_b, wexp_b)
            batch_epilogue(b, dens)
            dens = []

    return nc


def shard_inputs(dec_hidden, enc_output, W_score, b_score, v):
    """Full inputs -> per-core input maps (host-side layout staging)."""
    dec_hidden = np.ascontiguousarray(dec_hidden, dtype=np.float32)
    W_score = np.asarray(W_score, dtype=np.float32)
    # W_dec tiled partition-major: [p, dt, kk, m] with value W[kk*128+p, dt*128+m]
    wd_t = np.ascontiguousarray(
        W_score[:DEC]
        .reshape(DT, 128, DT, 128)
        .transpose(1, 2, 0, 3)
        .reshape(128, DT * DT * 128)
    ).astype(ml_dtypes.bfloat16)
    # W_enc tiled partition-major: [p, k, d]
    wenc8 = np.ascontiguousarray(
        (W_score[DEC:] * WSCALE).reshape(KT, 128, DEC).transpose(1, 0, 2)
    ).astype(ml_dtypes.float8_e4m3)
    b_mat = np.ascontiguousarray(
        np.asarray(b_score, dtype=np.float32).reshape(DT, 128).T
    )
    v_mat = (
        np.ascontiguousarray(np.asarray(v, dtype=np.float32).reshape(DT, 128).T)
        .astype(ml_dtypes.bfloat16)
    )
    eye = np.eye(128, dtype=np.float32)

    in_maps = []
    for core in range(N_CORES):
        sl = slice(core * BL, (core + 1) * BL)
        # (L, BL, 2E) -> (BL, 2E, L)
        enc_t = np.ascontiguousarray(
            np.asarray(enc_output[:, sl, :], dtype=np.float32).transpose(1, 2, 0)
        )
        # fp8 chunk-major partition-tiled: [b, c, half, p, k, l]
        enc8_t = np.ascontiguousarray(
            enc_t.reshape(BL, 2, KH, 128, NLC, LC).transpose(0, 4, 1, 3, 2, 5)
        ).astype(ml_dtypes.float8_e4m3)
        # bf16 full-L partition-tiled: [b, half, p, k, l]
        encb_t = np.ascontiguousarray(
            enc_t.reshape(BL, 2, KH, 128, L).transpose(0, 1, 3, 2, 4)
        ).astype(ml_dtypes.bfloat16)
        # (BL, DEC) -> [p, kt, b]
        dec_kpb = (
            np.ascontiguousarray(
                dec_hidden[sl].T.reshape(DT, 128, BL).transpose(1, 0, 2)
            )
            .astype(ml_dtypes.bfloat16)
        )
        in_maps.append(
            {
                "enc8_t": enc8_t,
                "encb_t": encb_t,
                "ones": np.ones((1, 128), dtype=np.float32),
                "dec_kpb": dec_kpb,
                "wenc8": wenc8,
                "w_dec": wd_t,
                "b_mat": b_mat,
                "v_mat": v_mat,
                "eye": eye,
            }
        )
    return in_maps


_NC_CACHE = None


def kernel(dec_hidden, enc_output, W_score, b_score, v):
    global _NC_CACHE
    if _NC_CACHE is None:
        _NC_CACHE = build_nc()
    nc = _NC_CACHE
    in_maps = shard_inputs(dec_hidden, enc_output, W_score, b_score, v)
    res = run_bass_kernel_spmd(nc, in_maps, list(range(N_CORES)))
    return np.concatenate([res.results[i]["att"] for i in range(N_CORES)], axis=0)
